# revision 1
# baseline (speedup 1.0000x reference)
"""CAM (channel attention) kernel for Trainium2, 8-core data-parallel over batch.

Per batch item (one per NeuronCore):
    energy   = Q @ K^T                     (C x C, contract over N)
    att      = softmax(max(energy) - energy) = softmax(-energy)   (shift-invariant)
    out      = gamma * (att @ V) + V

Per-core layout (q,k,v: [C=512, N=4096] f32 in DRAM):
  - q,k streamed in as f32 chunks (SWDGE), cast to bf16 on DVE/ACT, then
    DMA-xbar block-transposes ([128,1024] -> [128,8,128]) into per-n-group
    packed qT/kT tensors, split across both HWDGE rings. (SWDGE cast-during-
    DMA would be cheaper but signals completion before data lands on HW.)
  - energy accumulates in 4 PSUM banks ([128c, 512d]) over 32 n-chunks; the
    rhs spans all four kT c-tiles via a strided 3D access pattern (N=512/MM).
  - softmax over the free dim: DVE min, ACT exp(bias=rowmin, scale=-1) with
    fused row-sum, DVE reciprocal; gamma is folded into the normalization
    scale so matmul2 directly produces gamma*(att@V); att is bf16.
  - att transposed via PE (identity matmul) to attT (bf16).
  - matmul2 (bf16): for each 512-wide output chunk, v slices are cast
    f32->bf16 on DVE/ACT, 4 accumulating matmuls per c-tile, epilogue
    out = psum + v (f32) on DVE, stored per [128,512].
"""

import numpy as np

B, C, H, W = 8, 512, 64, 64
N = H * W  # 4096
P = 128
CT = C // P  # 4 c-tiles
NJ = N // P  # 32 n-chunks
NO = N // 512  # 8 output column chunks

_nc_cache: dict = {}


def _body(nc, tc, cfg):
    from contextlib import ExitStack

    import concourse.mybir as mybir
    from concourse.bass import ts
    from concourse.masks import make_identity

    cfg = cfg or {}
    front = cfg.get("front", "f32_cast")  # f32_cast | cast_dma (cast-DMA races on HW)
    do = lambda phase: phase not in cfg.get("skip", ())

    dt = mybir.dt
    f32, bf16 = dt.float32, dt.bfloat16
    X = mybir.AxisListType.X

    qa = nc.kio["q"].ap().rearrange("(a p) w -> a p w", p=P)
    ka = nc.kio["k"].ap().rearrange("(a p) w -> a p w", p=P)
    va = nc.kio["v"].ap().rearrange("(a p) w -> a p w", p=P)
    ga = nc.kio["gamma"].ap()
    oa = nc.kio["out"].ap().rearrange("(a p) w -> a p w", p=P)
    oa_p = nc.kio["out"].ap().rearrange("(a p) w -> p a w", p=P)

    hwdge = [nc.sync, nc.scalar]

    with ExitStack() as ctx:
        ep = ctx.enter_context

        p_nat = ep(tc.tile_pool(name="nat", bufs=5 if front == "f32_cast" else (cfg.get("nat_bufs", 6) if not cfg.get("ng") else 3)))
        p_T = ep(tc.tile_pool(name="pT", bufs=1))
        p_vf = ep(tc.tile_pool(name="vf", bufs=CT))
        p_att = ep(tc.tile_pool(name="att", bufs=CT))
        p_attT = ep(tc.tile_pool(name="attT", bufs=CT))
        p_small = ep(tc.tile_pool(name="small", bufs=2))
        p_misc = ep(tc.tile_pool(name="misc", bufs=1))
        p_vbs = ep(tc.tile_pool(name="vbs", bufs=2))
        p_es = ep(tc.tile_pool(name="es", bufs=2))

        # gamma broadcast across partitions: [1,1] DRAM -> [128,1] SBUF
        g128 = p_misc.tile([P, 1], f32)
        nc.sync.dma_start(g128[:], ga.broadcast_to([P, 1]))

        ident = p_misc.tile([P, P], bf16)
        make_identity(nc, ident[:])

        # packed transposed tensors, one tile per n-group of GJ chunks:
        # qT[g] is [p, c-tile, jj, 128] with j = g*GJ + jj
        NG = cfg.get("ng", 4)  # n-groups
        GJ = NJ // NG  # chunks per group
        GW = GJ * P  # columns per group chunk (1024)
        qT = [
            p_T.tile([P, CT, GJ, P], bf16, tag="qT", bufs=2, name=f"qT{g}")
            for g in range(NG)
        ]
        # kT layout [p, jj, c, 128]: each n-chunk's rhs is contiguous [128, 512]
        kT = [
            p_T.tile([P, GJ, CT, P], bf16, tag="kT", bufs=2, name=f"kT{g}")
            for g in range(NG)
        ]

        # q,k transposed via DMA xbar (tq='pe' routes q through the PE instead)
        tq = cfg.get("tq", "xbar")  # xbar | pe

        att = []
        v_f = []
        with tc.tile_pool(name="energy", bufs=CT, space="PSUM") as p_energy:
            e_ps = [
                p_energy.tile([P, 512], f32, tag="e", name=f"e{c}")
                for c in range(CT)
            ]

            # stream q,k in [128, GW] chunks (cast-DMA) -> transpose
            if do("loads_qk"):
                with tc.tile_pool(name="ptp", bufs=4, space="PSUM") as p_ptp:
                    for g in range(NG):
                        for c in range(CT):
                            qn = p_nat.tile(
                                [P, GW], bf16, tag="qn", name=f"qn{g}{c}"
                            )
                            kn = p_nat.tile(
                                [P, GW], bf16, tag="kn", name=f"kn{g}{c}"
                            )
                            if front == "cast_dma":
                                nc.gpsimd.dma_start(qn[:], qa[c][:, ts(g, GW)])
                                nc.gpsimd.dma_start(kn[:], ka[c][:, ts(g, GW)])
                            else:
                                qn32 = p_nat.tile(
                                    [P, GW], f32, tag="qn32", name=f"qn32_{g}{c}"
                                )
                                nc.sync.dma_start(qn32[:], qa[c][:, ts(g, GW)])
                                nc.vector.tensor_copy(qn[:], qn32[:])
                                kn32 = p_nat.tile(
                                    [P, GW], f32, tag="kn32", name=f"kn32_{g}{c}"
                                )
                                nc.gpsimd.dma_start(kn32[:], ka[c][:, ts(g, GW)])
                                nc.scalar.copy(kn[:], kn32[:])
                            if not do("tpose"):
                                continue
                            if tq == "pe":
                                for jj in range(GJ):
                                    ptp = p_ptp.tile([P, P], bf16)
                                    nc.tensor.transpose(
                                        ptp[:], qn[:, ts(jj, P)], ident[:]
                                    )
                                    if (c * GJ + jj) % 2 == 0:
                                        nc.vector.tensor_copy(
                                            qT[g][:, c, jj, :], ptp[:]
                                        )
                                    else:
                                        nc.scalar.copy(
                                            qT[g][:, c, jj, :], ptp[:]
                                        )
                            else:
                                nc.scalar.dma_start(
                                    qT[g][:, c], qn[:], transpose=True
                                )
                            nc.scalar.dma_start(
                                kT[g][:, :, c, :], kn[:], transpose=True
                            )

            # v loads (f32)
            if do("loads_v"):
                v_eng = cfg.get("v_eng", "hwdge")
                for c in range(CT):
                    vf = p_vf.tile([P, N], f32, tag="vf", name=f"vf{c}")
                    if c % 2 == 0:
                        nc.gpsimd.dma_start(vf[:], va[c])
                    else:
                        nc.sync.dma_start(vf[:], va[c])
                    v_f.append(vf)

            if not (do("loads_qk") and do("tpose") and do("mm1")):
                return

            # matmul1: energy[c] += qT[:,c,j,:].T @ kT[:,:,j,:]  (N=512)
            for g in range(NG):
                for jj in range(GJ):
                    for c in range(CT):
                        nc.tensor.matmul(
                            e_ps[c][:],
                            qT[g][:, c, jj, :],
                            kT[g][:, jj],
                            start=(g == 0 and jj == 0),
                            stop=(g == NG - 1 and jj == GJ - 1),
                        )

            if cfg.get("dump_qT"):
                ob = nc.kio["out"].ap().bitcast(bf16)
                ob = ob.rearrange("(a p) w -> a p w", p=P)
                hwdge[0].dma_start(
                    ob[0][:, 0:4096], qT[0].rearrange("p a b c -> p (a b c)")
                )
                hwdge[1].dma_start(
                    ob[1][:, 0:4096], kT[0].rearrange("p a b c -> p (a b c)")
                )
                return

            if cfg.get("dump_energy"):
                for c in range(CT):
                    ed = p_es.tile([P, 512], f32, tag="ed", name=f"ed{c}")
                    nc.vector.tensor_copy(ed[:], e_ps[c][:])
                    nc.sync.dma_start(oa[c][:, 0:512], ed[:])
                return

            # softmax(-energy) rows, gamma folded into the normalization
            for c in range(CT):
                rowmin = p_small.tile([P, 1], f32)
                nc.vector.tensor_reduce(
                    rowmin[:], e_ps[c][:], axis=X, op=mybir.AluOpType.min
                )
                pexp = p_att.tile([P, 512], bf16, tag="att", name=f"att{c}")
                rowsum = p_small.tile([P, 1], f32)
                nc.scalar.activation(
                    pexp[:],
                    e_ps[c][:],
                    mybir.ActivationFunctionType.Exp,
                    bias=rowmin[:, 0:1],
                    scale=-1.0,
                    accum_out=rowsum[:, 0:1],
                )
                recip = p_small.tile([P, 1], f32)
                nc.vector.reciprocal(recip[:], rowsum[:])
                srow = p_small.tile([P, 1], f32)
                nc.vector.tensor_scalar_mul(srow[:], recip[:], g128[:, 0:1])
                nc.vector.tensor_scalar_mul(pexp[:], pexp[:], srow[:, 0:1])
                att.append(pexp)

        if not do("mm2"):
            return

        # transpose att (bf16) via PE into attT[d][:, c-block]
        attT = []
        with tc.tile_pool(name="pst", bufs=2, space="PSUM") as p_pst:
            for d in range(CT):
                at = p_attT.tile([P, C], bf16, tag="attT", name=f"attT{d}")
                for c in range(CT):
                    pst = p_pst.tile([P, P], bf16)
                    nc.tensor.transpose(pst[:], att[c][:, ts(d, P)], ident[:])
                    nc.vector.tensor_copy(at[:, ts(c, P)], pst[:])
                attT.append(at)

        # matmul2 (bf16): psum = gamma*(att @ V); epilogue adds v (f32)
        with tc.tile_pool(name="ps2", bufs=4, space="PSUM") as p_ps2:
            for no in range(NO):
                vbs = []
                for d in range(CT):
                    vb = p_vbs.tile(
                        [P, 512], bf16, tag=f"vb{d}", name=f"vb{d}_{no}"
                    )
                    if d % 2 == 0:
                        nc.vector.tensor_copy(vb[:], v_f[d][:, ts(no, 512)])
                    else:
                        nc.scalar.copy(vb[:], v_f[d][:, ts(no, 512)])
                    vbs.append(vb)
                es4 = p_es.tile([P, CT, 512], f32)
                for c in range(CT):
                    ps2 = p_ps2.tile([P, 512], f32)
                    for d in range(CT):
                        nc.tensor.matmul(
                            ps2[:],
                            attT[d][:, ts(c, P)],
                            vbs[d][:],
                            start=(d == 0),
                            stop=(d == CT - 1),
                        )
                    nc.vector.tensor_add(
                        es4[:, c, :], ps2[:], v_f[c][:, ts(no, 512)]
                    )
                nc.sync.dma_start(oa_p[:, :, ts(no, 512)], es4[:])


def build(repeat=1, cfg=None, loop_n=None):
    import concourse.mybir as mybir
    import concourse.tile as tile
    from concourse import bacc

    dt = mybir.dt
    nc = bacc.Bacc("TRN2", target_bir_lowering=False, debug=False)
    nc.kio = {}
    for name in ("q", "k", "v"):
        nc.kio[name] = nc.dram_tensor(
            name, [C, N], dt.float32, kind="ExternalInput"
        )
    nc.kio["gamma"] = nc.dram_tensor(
        "gamma", [1, 1], dt.float32, kind="ExternalInput"
    )
    nc.kio["out"] = nc.dram_tensor(
        "out", [C, N], dt.float32, kind="ExternalOutput"
    )
    with tile.TileContext(nc) as tc:
        if loop_n is not None:
            with tc.For_i(0, loop_n, 1):
                _body(nc, tc, cfg)
        else:
            for _ in range(repeat):
                _body(nc, tc, cfg)
    nc.compile()
    return nc


def _get_nc():
    if "nc" not in _nc_cache:
        _nc_cache["nc"] = build(repeat=1)
    return _nc_cache["nc"]


def make_in_maps(q, k, v, gamma):
    q = np.ascontiguousarray(np.asarray(q, dtype=np.float32).reshape(B, C, N))
    k = np.ascontiguousarray(np.asarray(k, dtype=np.float32).reshape(B, C, N))
    v = np.ascontiguousarray(np.asarray(v, dtype=np.float32).reshape(B, C, N))
    g = np.asarray(gamma, dtype=np.float32).reshape(1, 1)
    return [
        {"q": q[i], "k": k[i], "v": v[i], "gamma": g} for i in range(B)
    ]


def kernel(q, k, v, gamma):
    from concourse import bass_utils

    nc = _get_nc()
    in_maps = make_in_maps(q, k, v, gamma)
    res = bass_utils.run_bass_kernel_spmd(nc, in_maps, core_ids=list(range(B)))
    out = np.stack([res.results[i]["out"] for i in range(B)])
    return out.reshape(B, C, H, W).astype(np.float32, copy=False)



# revision 2
# speedup vs baseline: 1.0563x; 1.0563x over previous
"""CAM (channel attention) kernel for Trainium2, 8-core data-parallel over batch.

Per batch item (one per NeuronCore):
    energy   = Q @ K^T                     (C x C, contract over N)
    att      = softmax(max(energy) - energy) = softmax(-energy)   (shift-invariant)
    out      = gamma * (att @ V) + V

Design notes (HW-measured on this part):
  - q,k streamed as [128,1024] f32 chunks, BOTH on the sync HWDGE ring
    (SWDGE/gpsimd DMA has ~2us completion latency and its Pool-side issue
    blocks on pool-recycle waits, which serializes the whole phase-1
    pipeline -- measured 142.6us -> 113.8us from this change alone).
  - transposes on the PE as f32 identity matmuls (2 cyc/row); the
    PSUM->SBUF drain (DVE for q, ACT for k) casts f32->bf16, so there are
    no standalone cast ops. DMA-xbar transposes measured far slower here
    (218us kernel) and the PE sequencer's ~80ns/instruction issue rate is
    the real constraint: keeping f32 transposes also keeps mm1/mm2 as
    single self-loading InstMatmults (863 vs 1069 PE BIR instructions).
  - energy accumulates in 4 PSUM banks ([128c, 512d]) over 32 n-chunks;
    mm1 rhs is a strided [128, 4, 128] spanning the kT c-tiles (measured
    as fast as contiguous).
  - softmax over the free dim: DVE min, ACT exp(bias=rowmin, scale=-1)
    with fused row-sum, DVE reciprocal; gamma is folded into the
    normalization scale and the identity is added to the diagonal so that
    A = gamma*att + I and out = A @ V directly -- no f32 "+v" epilogue
    (costs one bf16 rounding of v, ~0.3% rel err at gamma=0).
  - A transposed via PE into attT before the v loads are emitted, so the
    DVE drain stream is not queued behind v-paced work.
  - v loaded as 8 column-blocks [128, 4, 512] (1 MiB DMAs, alternating
    sync/gpsimd), gated behind the q/k staging pipeline by sharing its
    tile-pool tag (a data dependency; ring FIFO alone does not hold the
    v transfers back). vb casts on ACT.
  - matmul2 (bf16): per 512-col chunk, 4 accumulating matmuls per c-tile;
    psum drained to SBUF on DVE and stored per [128, 4, 512] (1 MiB DMAs).
"""

import numpy as np

B, C, H, W = 8, 512, 64, 64
N = H * W  # 4096
P = 128
CT = C // P  # 4 c-tiles
NJ = N // P  # 32 n-chunks
NG = 4  # n-groups
GJ = NJ // NG  # 8 chunks per group
GW = GJ * P  # 1024 columns per group chunk
NO = N // 512  # 8 output column chunks

_nc_cache: dict = {}


def _body(nc, tc, cfg):
    from contextlib import ExitStack

    import concourse.mybir as mybir
    from concourse.bass import ts
    from concourse.masks import make_identity

    cfg = cfg or {}
    do = lambda phase: phase not in cfg.get("skip", ())

    dt = mybir.dt
    f32, bf16 = dt.float32, dt.bfloat16
    X = mybir.AxisListType.X

    qa = nc.kio["q"].ap().rearrange("(a p) w -> a p w", p=P)
    ka = nc.kio["k"].ap().rearrange("(a p) w -> a p w", p=P)
    va_p = nc.kio["v"].ap().rearrange("(a p) w -> p a w", p=P)
    ga = nc.kio["gamma"].ap()
    oa_p = nc.kio["out"].ap().rearrange("(a p) w -> p a w", p=P)

    with ExitStack() as ctx:
        ep = ctx.enter_context

        p_nat = ep(tc.tile_pool(name="nat", bufs=5))
        p_T = ep(tc.tile_pool(name="pT", bufs=1))
        p_att = ep(tc.tile_pool(name="att", bufs=CT))
        p_attT = ep(tc.tile_pool(name="attT", bufs=CT))
        p_small = ep(tc.tile_pool(name="small", bufs=2))
        p_misc = ep(tc.tile_pool(name="misc", bufs=1))
        p_vb = ep(tc.tile_pool(name="vb", bufs=NO))
        p_es = ep(tc.tile_pool(name="es", bufs=2))

        # gamma broadcast across partitions: [1,1] DRAM -> [128,1] SBUF
        # (on the ACT HWDGE ring, which carries no other DMA traffic)
        g128 = p_misc.tile([P, 1], f32)
        nc.scalar.dma_start(g128[:], ga.broadcast_to([P, 1]))

        ident = p_misc.tile([P, P], bf16)
        make_identity(nc, ident[:])
        ident32 = p_misc.tile([P, P], f32)
        make_identity(nc, ident32[:])

        # packed transposed tensors, one tile per n-group of GJ chunks:
        # qT[g] is [p, c-tile, jj, 128]; kT[g] is [p, jj, c-tile, 128] so
        # each n-chunk's mm1 rhs is a [128, 512] spanning all 4 c-tiles.
        qT = [
            p_T.tile([P, CT, GJ, P], bf16, tag="qT", bufs=2, name=f"qT{g}")
            for g in range(NG)
        ]
        kT = [
            p_T.tile([P, GJ, CT, P], bf16, tag="kT", bufs=2, name=f"kT{g}")
            for g in range(NG)
        ]

        att = []
        with tc.tile_pool(name="energy", bufs=CT, space="PSUM") as p_energy:
            e_ps = [
                p_energy.tile([P, 512], f32, tag="e", name=f"e{c}")
                for c in range(CT)
            ]

            if do("loads_qk"):
                with tc.tile_pool(name="ptp", bufs=4, space="PSUM") as p_ptp:
                    for g in range(NG):
                        for c in range(CT):
                            qn32 = p_nat.tile(
                                [P, GW], f32, tag="qn32", name=f"qn32_{g}{c}"
                            )
                            nc.sync.dma_start(qn32[:], qa[c][:, ts(g, GW)])
                            kn32 = p_nat.tile(
                                [P, GW], f32, tag="kn32", name=f"kn32_{g}{c}"
                            )
                            nc.sync.dma_start(kn32[:], ka[c][:, ts(g, GW)])

                            if not do("tpose"):
                                continue
                            # f32 transposes on the PE (2 cyc/row); the
                            # PSUM->SBUF drain does the f32->bf16 cast, so
                            # there are no standalone cast ops at all.
                            # PSUM is 4B/elem: 4 tiles per staging bank.
                            HJ = GJ // 2
                            for h in range(2):
                                pq = p_ptp.tile([P, HJ, P], f32, tag="pq", bufs=2)
                                for j in range(HJ):
                                    nc.tensor.transpose(
                                        pq[:, j, :],
                                        qn32[:, ts(h * HJ + j, P)],
                                        ident32[:],
                                    )
                                nc.vector.tensor_copy(
                                    qT[g][:, c, ts(h, HJ), :], pq[:]
                                )
                                pk = p_ptp.tile([P, HJ, P], f32, tag="pk", bufs=2)
                                for j in range(HJ):
                                    nc.tensor.transpose(
                                        pk[:, j, :],
                                        kn32[:, ts(h * HJ + j, P)],
                                        ident32[:],
                                    )
                                nc.scalar.copy(
                                    kT[g][:, ts(h, HJ), c, :], pk[:]
                                )

                        if not (do("tpose") and do("mm1")):
                            continue
                        # matmul1 for this group:
                        # energy[c] += qT[g][:,c,jj,:].T @ kT[g][:,jj]
                        for jj in range(GJ):
                            for c in range(CT):
                                nc.tensor.matmul(
                                    e_ps[c][:],
                                    qT[g][:, c, jj, :],
                                    kT[g][:, jj],
                                    start=(g == 0 and jj == 0),
                                    stop=(g == NG - 1 and jj == GJ - 1),
                                )

            if not (do("loads_qk") and do("tpose") and do("mm1")):
                return
            if not do("softmax"):
                return

            # softmax(-energy) rows; build A = gamma*att + I in bf16
            for c in range(CT):
                rowmin = p_small.tile([P, 1], f32)
                nc.vector.tensor_reduce(
                    rowmin[:], e_ps[c][:], axis=X, op=mybir.AluOpType.min
                )
                pexp = p_att.tile([P, 512], bf16, tag="att", name=f"att{c}")
                rowsum = p_small.tile([P, 1], f32)
                nc.scalar.activation(
                    pexp[:],
                    e_ps[c][:],
                    mybir.ActivationFunctionType.Exp,
                    bias=rowmin[:, 0:1],
                    scale=-1.0,
                    accum_out=rowsum[:, 0:1],
                )
                recip = p_small.tile([P, 1], f32)
                nc.vector.reciprocal(recip[:], rowsum[:])
                srow = p_small.tile([P, 1], f32)
                nc.vector.tensor_scalar_mul(srow[:], recip[:], g128[:, 0:1])
                nc.vector.tensor_scalar_mul(pexp[:], pexp[:], srow[:, 0:1])
                nc.vector.tensor_add(
                    pexp[:, ts(c, P)], pexp[:, ts(c, P)], ident[:]
                )
                att.append(pexp)

        # transpose A (bf16) via PE into attT[d][:, c-block] -- emitted
        # before the v loads so the DVE attT copies aren't queued behind
        # the v-arrival-paced vb casts (in-order engine streams)
        attT = []
        with tc.tile_pool(name="pst", bufs=2, space="PSUM") as p_pst:
            for d in range(CT):
                at = p_attT.tile([P, C], bf16, tag="attT", name=f"attT{d}")
                for c in range(CT):
                    pst = p_pst.tile([P, P], bf16)
                    nc.tensor.transpose(pst[:], att[c][:, ts(d, P)], ident[:])
                    nc.vector.tensor_copy(at[:, ts(c, P)], pst[:])
                attT.append(at)

        # v loads: all 8 column-blocks, gated behind the q/k staging pipeline
        # via the shared kn32 tag (dependency, not ring order). vb casts all
        # on ACT so the DVE es-copy stream isn't blocked behind them.
        vb = []
        if do("loads_v"):
            for no in range(NO):
                v32 = p_nat.tile([P, CT, 512], f32, tag="kn32", name=f"v32_{no}")
                eng = nc.sync if no % 2 == 0 else nc.gpsimd
                eng.dma_start(v32[:], va_p[:, :, ts(no, 512)])
                vbt = p_vb.tile([P, CT, 512], bf16, tag="vb", name=f"vb{no}")
                nc.scalar.copy(vbt[:], v32[:])
                vb.append(vbt)

        if not do("mm2"):
            return

        # matmul2 (bf16): psum = A @ V = gamma*(att @ V) + V
        with tc.tile_pool(name="ps2", bufs=4, space="PSUM") as p_ps2:
            for no in range(NO):
                es = p_es.tile([P, CT, 512], f32, tag="es", name=f"es{no}")
                for c in range(CT):
                    ps2 = p_ps2.tile([P, 512], f32)
                    for d in range(CT):
                        nc.tensor.matmul(
                            ps2[:],
                            attT[d][:, ts(c, P)],
                            vb[no][:, d],
                            start=(d == 0),
                            stop=(d == CT - 1),
                        )
                    nc.vector.tensor_copy(es[:, c], ps2[:])
                eng = nc.sync if no % 2 == 0 else nc.gpsimd
                eng.dma_start(oa_p[:, :, ts(no, 512)], es[:])


def build(repeat=1, cfg=None, loop_n=None):
    import concourse.mybir as mybir
    import concourse.tile as tile
    from concourse import bacc

    dt = mybir.dt
    nc = bacc.Bacc("TRN2", target_bir_lowering=False, debug=False)
    nc.kio = {}
    for name in ("q", "k", "v"):
        nc.kio[name] = nc.dram_tensor(
            name, [C, N], dt.float32, kind="ExternalInput"
        )
    nc.kio["gamma"] = nc.dram_tensor(
        "gamma", [1, 1], dt.float32, kind="ExternalInput"
    )
    nc.kio["out"] = nc.dram_tensor(
        "out", [C, N], dt.float32, kind="ExternalOutput"
    )
    with tile.TileContext(nc) as tc:
        if loop_n is not None:
            with tc.For_i(0, loop_n, 1):
                _body(nc, tc, cfg)
        else:
            for _ in range(repeat):
                _body(nc, tc, cfg)
    nc.compile()
    return nc


def _get_nc():
    if "nc" not in _nc_cache:
        _nc_cache["nc"] = build(repeat=1)
    return _nc_cache["nc"]


def make_in_maps(q, k, v, gamma):
    q = np.ascontiguousarray(np.asarray(q, dtype=np.float32).reshape(B, C, N))
    k = np.ascontiguousarray(np.asarray(k, dtype=np.float32).reshape(B, C, N))
    v = np.ascontiguousarray(np.asarray(v, dtype=np.float32).reshape(B, C, N))
    g = np.asarray(gamma, dtype=np.float32).reshape(1, 1)
    return [
        {"q": q[i], "k": k[i], "v": v[i], "gamma": g} for i in range(B)
    ]


def kernel(q, k, v, gamma):
    from concourse import bass_utils

    nc = _get_nc()
    in_maps = make_in_maps(q, k, v, gamma)
    res = bass_utils.run_bass_kernel_spmd(nc, in_maps, core_ids=list(range(B)))
    out = np.stack([res.results[i]["out"] for i in range(B)])
    return out.reshape(B, C, H, W).astype(np.float32, copy=False)
